# revision 76
# baseline (speedup 1.0000x reference)
"""Trainium2 Bass kernel for nn_MultiHeadAttention_75548474736720.

Linear-attention-style multi-head attention with causal prefix sums:
  qh/kh/vh = projections, ph = split_heads(p)
  A1 = elu(qh ph^T) + 1                       [t,s] per (b,h)
  U  = (tril(qh kh^T)/idx) @ A1 ; W = softmax(U)
  out[t] = (1/(t+1)) sum_{s<=t} (W[t]·A1[s]) vh[s] ; reshape @ wc + b

Sharding: 8 cores = (batch b in 0..1) x (head-group hg in 0..3, 4 heads each).
Each core computes its 4 heads end-to-end (wq/wk/wv column-sliced, wc
row-sliced) and returns a partial [S, Dm] output; host sums partials per batch.

All matmuls run in bf16 (f32 PSUM accumulation).  Key algebraic tricks:
  - exp without max-subtraction (scaled U bounded ~|19| for this family)
  - first prefix sum via a running accumulator E_P[c,j] = sum_{s<256P}
    kh[s,c] A1[s,j] per head: U for a 256-row pair = qh @ E_P plus two
    masked scoreT strips, instead of the O(S*S) masked-score sweep
  - W^T produced DIRECTLY: U^T[j,t] = E^T @ (qh/(t+1)) + A1^T @ strips has
    both contractions j-/s-partitioned already, so no W transposes or
    PSUM->SBUF copies; the per-t softmax scale is folded into a pre-scaled
    qh copy (qhTs) and mask*1/(t+1) strip constants (maskinvE)
  - softmax denominator recovered per t-block by N=1 ones-column matmuls
    against W^T (emitted after the out-group closes: interleaving a
    start=True sub-range group inside an open psum group corrupts it)
  - second prefix sum via a running accumulator C[j,d] = sum_{s<t0} A1[s,j]
    vh[s,d] per head: out-block i = W-block @ C + tril(W A1_i^T) @ vh_i,
    which is O(S*S*D) instead of O(S*S*S) for the explicit S2 matrix
"""

import sys

sys.path.insert(0, "/opt/trn_rl_repo")

import ml_dtypes
import numpy as np

import concourse.bass as bass  # noqa: F401  (registers AP machinery)
import concourse.mybir as mybir
from concourse import bacc
from concourse.tile import TileContext
from concourse.bass_utils import run_bass_kernel_spmd

F32 = mybir.dt.float32
BF16 = mybir.dt.bfloat16
ACTF = mybir.ActivationFunctionType
ALU = mybir.AluOpType
NPBF = ml_dtypes.bfloat16

B, S, DM, H = 2, 1024, 1024, 16
D = DM // H            # 64, head dim
HG = 4                 # heads per core
DL = HG * D            # 256, local dm slice
NB = S // 128          # 8 s-blocks
NORM_D = 0.125         # 1/sqrt(D)

# compact SqT layout: per s-block m, columns stored from t = 512*(m//4)
SQBASE = [0, 1024, 2048, 3072, 4096, 4608, 5120, 5632]  # total 6144

DEBUG = False


def _sq_off(m, t0):
    return SQBASE[m] + t0 - 512 * (m // 4)


def _build_program():
    nc = bacc.Bacc(None, target_bir_lowering=False)

    qT_in = nc.declare_dram_parameter("qT", [DM, S], BF16, isOutput=False)
    kT_in = nc.declare_dram_parameter("kT", [DM, S], BF16, isOutput=False)
    vT_in = nc.declare_dram_parameter("vT", [DM, S], BF16, isOutput=False)
    pT_in = nc.declare_dram_parameter("pT", [DL, S], BF16, isOutput=False)
    wq_in = nc.declare_dram_parameter("wq", [DM, DL], BF16, isOutput=False)
    wk_in = nc.declare_dram_parameter("wk", [DM, DL], BF16, isOutput=False)
    wv_in = nc.declare_dram_parameter("wv", [DM, DL], BF16, isOutput=False)
    wc_in = nc.declare_dram_parameter("wc", [DL, S], BF16, isOutput=False)
    wqb_in = nc.declare_dram_parameter("wqb", [128, 2], F32, isOutput=False)
    wkb_in = nc.declare_dram_parameter("wkb", [128, 2], F32, isOutput=False)
    wvb_in = nc.declare_dram_parameter("wvb", [1, DL], BF16, isOutput=False)
    ones_in = nc.declare_dram_parameter("ones1", [1, 128], BF16, isOutput=False)
    mask_in = nc.declare_dram_parameter("mask4", [4, 128, 512], BF16, isOutput=False)
    ident_in = nc.declare_dram_parameter("ident", [128, 128], BF16, isOutput=False)
    inv_in = nc.declare_dram_parameter("invidx", [128, NB], F32, isOutput=False)
    mie_in = nc.declare_dram_parameter("maskinvE", [4, 128, 384], F32, isOutput=False)
    invb_in = nc.declare_dram_parameter("invtbc", [128, S], F32, isOutput=False)
    out_d = nc.declare_dram_parameter("out", [S, DM], F32, isOutput=True)
    dbg = {}
    if DEBUG:
        dbg["qhT"] = nc.declare_dram_parameter("d_qhT", [128, 2 * S], F32, isOutput=True)
        dbg["vh"] = nc.declare_dram_parameter("d_vh", [128, NB * DL], F32, isOutput=True)
        dbg["a1"] = nc.declare_dram_parameter("d_a1", [128, NB * S], F32, isOutput=True)
        dbg["sqT"] = nc.declare_dram_parameter("d_sqT", [128, 6144], F32, isOutput=True)
        dbg["wtT"] = nc.declare_dram_parameter("d_wtT", [128, NB * S], F32, isOutput=True)
        dbg["oT"] = nc.declare_dram_parameter("d_oT", [64, HG * S], F32, isOutput=True)
        dbg["den"] = nc.declare_dram_parameter("d_den", [128, NB], F32, isOutput=True)

    with TileContext(nc) as tc:
        with tc.tile_pool(name="persist", bufs=1) as cp, \
             tc.tile_pool(name="ppm", bufs=2, space="PSUM") as ppm, \
             tc.tile_pool(name="ppt", bufs=1, space="PSUM") as ppt:

            mask = cp.tile([128, 4, 512], BF16)
            ident = cp.tile([128, 128], BF16)
            invidx = cp.tile([128, NB], F32)
            wqb = cp.tile([128, 2], F32)
            wkb = cp.tile([128, 2], F32)
            wvb = cp.tile([1, DL], BF16)
            ones1 = cp.tile([1, 128], BF16)
            pTt = cp.tile([128, 2, S], BF16)
            qhT = cp.tile([128, 2, S], BF16)
            khT = cp.tile([128, 2, S], BF16)
            vh = cp.tile([128, NB, DL], BF16)
            oT = cp.tile([128, 2, S], BF16)
            # wc stored per head-pair: wct[:, g, :] = wc rows [g*128:(g+1)*128];
            # loaded up front so the output projection never waits on DMA
            wct = cp.tile([128, 2, S], BF16)

            # ---------------- projections ----------------
            # DMA issue on SP costs ~0.5us per descriptor, so the inputs the
            # first matmuls need go first, split 4-ways for queue parallelism;
            # constants (masks, wc, p) follow.  The v projection runs inside
            # the attention phase (interleaved with head 0's A1/SqT) so its
            # tiles live in a separate pool that outlives the q/k one.
            vp_cm = tc.tile_pool(name="vproj", bufs=1)
            vp = vp_cm.__enter__()
            wvt = vp.tile([128, NB, DL], BF16)
            vTt = vp.tile([128, NB, S], BF16)
            with tc.tile_pool(name="proj", bufs=1) as jp:
                wqt = jp.tile([128, NB, DL], BF16)
                wkt = jp.tile([128, NB, DL], BF16)
                qTt = jp.tile([128, NB, S], BF16)
                kTt = jp.tile([128, NB, S], BF16)
                def ldw(wt_, wsrc, kb, n=2):
                    nc.sync.dma_start(
                        out=wt_[:, kb:kb + n, :],
                        in_=wsrc[kb * 128:(kb + n) * 128, :].rearrange(
                            "(a p) d -> p a d", p=128))

                def ldx(xt_, xsrc, kb, n=2):
                    nc.sync.dma_start(
                        out=xt_[:, kb:kb + n, :],
                        in_=xsrc[kb * 128:(kb + n) * 128, :].rearrange(
                            "(a p) t -> p a t", p=128))

                # transfers are serialized on the DMA path, so issue in the
                # exact order the kb-outer projection matmuls consume them,
                # single-block input chunks so PE streams behind the DMA
                ldw(wqt, wq_in, 0, 4)
                ldx(qTt, qT_in, 0, 1); ldx(qTt, qT_in, 1, 1)
                ldw(wqt, wq_in, 4, 4)
                nc.sync.dma_start(out=wqb[:], in_=wqb_in[:])
                for kb in range(2, NB):
                    ldx(qTt, qT_in, kb, 1)
                ldw(wkt, wk_in, 0, 4); ldw(wkt, wk_in, 4, 4)
                nc.sync.dma_start(out=wkb[:], in_=wkb_in[:])
                for kb in range(NB):
                    ldx(kTt, kT_in, kb, 1)
                nc.sync.dma_start(
                    out=pTt[:], in_=pT_in.rearrange("(g p) t -> p g t", p=128))
                nc.sync.dma_start(out=invidx[:], in_=inv_in[:])
                nc.sync.dma_start(
                    out=mask[:], in_=mask_in.rearrange("r p c -> p r c"))
                nc.sync.dma_start(out=ident[:], in_=ident_in[:])
                ldw(wvt, wv_in, 0, 4); ldw(wvt, wv_in, 4, 4)
                for kb in range(0, NB, 2):
                    ldx(vTt, vT_in, kb)
                nc.sync.dma_start(out=wvb[:], in_=wvb_in[:])
                nc.sync.dma_start(out=ones1[:], in_=ones_in[:])
                nc.sync.dma_start(
                    out=miE[:], in_=mie_in.rearrange("r p c -> p r c"))
                nc.sync.dma_start(out=invtbc[:], in_=invb_in[:])
                nc.sync.dma_start(
                    out=wct[:], in_=wc_in.rearrange("(a p) t -> p a t", p=128))

                # qhT[dm, t] = sum_c wq[c, dm] qT[c, t]  (+bias, * 1/sqrt(D))
                # kb-outer accumulation so PE consumes input chunks as the
                # (serialized) DMA stream lands them instead of waiting for
                # the full operand
                for wt_, xt_, dst, bias_t, scale in (
                    (wqt, qTt, qhT, wqb, NORM_D),
                    (wkt, kTt, khT, wkb, 1.0),
                ):
                    for g in range(2):
                        pss = [ppm.tile([128, 512], F32, tag="mm", name="ps_proj")
                               for _ in range(2)]
                        for kb in range(NB):
                            for n in range(2):
                                nc.tensor.matmul(
                                    pss[n][:], wt_[:, kb, g * 128:(g + 1) * 128],
                                    xt_[:, kb, n * 512:(n + 1) * 512],
                                    start=(kb == 0), stop=(kb == NB - 1))
                        for n in range(2):
                            nc.scalar.activation(
                                dst[:, g, n * 512:(n + 1) * 512], pss[n][:],
                                ACTF.Identity, bias=bias_t[:, g:g + 1], scale=scale)

                if DEBUG:
                    nc.sync.dma_start(out=dbg["qhT"].rearrange("p (a b) -> p a b", a=2),
                                      in_=qhT[:])

            # ---------------- attention (4 heads) ----------------
            # Pair-level software pipeline: A1/SqT for head h+1 are emitted
            # between U(h) and S2(h) so the in-order PE stream always has
            # independent matmuls to run while elementwise chains drain.
            with tc.tile_pool(name="attn", bufs=2) as ap, \
                 tc.tile_pool(name="scr", bufs=2) as sp:
                st = {}

                def gen_a1_sq(h):
                    """Generator: yields after each matmul unit so A1/SqT of
                    head h can be interleaved into head h-1's S2 phase (keeps
                    the in-order PE queue fed while elementwise chains drain).

                    A1 = elu(x)+1 = min(exp(x), 1) + relu(x); exp is safe
                    unclamped (|x| <= ~8 here).  The PSUM tile is released
                    after just two reads (exp on ACT, relu on DVE, which run
                    concurrently); min and the final add run on idle GPSIMD
                    from SBUF so the a1ps ring turns over fast."""
                    g, p0 = h // 2, (h % 2) * 64
                    a1 = ap.tile([128, NB, S], BF16, tag="a1", name="a1")
                    sqT = ap.tile([128, 6144], BF16, tag="sq", name="sqT")
                    st[h] = [a1, sqT]

                    def a1_unit(m, c):
                        ps = ppm.tile([128, 512], F32, tag="a1ps", bufs=3,
                                      name="ps_a1")
                        nc.tensor.matmul(
                            ps[:], qhT[p0:p0 + 64, g, m * 128:(m + 1) * 128],
                            pTt[p0:p0 + 64, g, c * 512:(c + 1) * 512],
                            start=True, stop=True)
                        e = sp.tile([128, 512], F32, tag="e", bufs=8, name="e")
                        nc.scalar.activation(e[:], ps[:], ACTF.Exp)
                        e1 = sp.tile([128, 512], F32, tag="e1", bufs=8, name="e1")
                        nc.gpsimd.tensor_scalar_min(e1[:], e[:], 1.0)
                        nc.vector.scalar_tensor_tensor(
                            a1[:, m, c * 512:(c + 1) * 512], ps[:], 0.0, e1[:],
                            ALU.max, ALU.add)

                    def sq_unit(m, n):
                        ps = ppm.tile([128, 512], F32, tag="mm", name="ps_sq")
                        nc.tensor.matmul(
                            ps[:], khT[p0:p0 + 64, g, m * 128:(m + 1) * 128],
                            qhT[p0:p0 + 64, g, n * 512:(n + 1) * 512],
                            start=True, stop=True)
                        dst = sqT[:, _sq_off(m, n * 512):_sq_off(m, n * 512) + 512]
                        if n == m // 4:
                            nc.vector.tensor_tensor(dst, ps[:], mask[:, m % 4, :], ALU.mult)
                        else:
                            nc.scalar.activation(dst, ps[:], ACTF.Copy)

                    # interleave A1 and sq units so consecutive A1 psum tiles
                    # (whose consumer chain is long) are spaced ~2 units apart
                    for m in range(NB):
                        a1_unit(m, 0)
                        yield
                        sq_unit(m, 0 if m < 4 else 1)
                        yield
                        a1_unit(m, 1)
                        yield
                        if m < 4:
                            sq_unit(m, 1)
                            yield
                    if DEBUG and h == 0:
                        nc.sync.dma_start(
                            out=dbg["a1"].rearrange("p (a b) -> p a b", a=NB), in_=a1[:])
                        nc.sync.dma_start(out=dbg["sqT"][:, :], in_=sqT[:])

                class GenState:
                    """Paces a gen_a1_sq generator across phases; tracks how
                    many units have been emitted so consumers can require
                    progress (U-block i needs all units for s-blocks <= i)."""
                    def __init__(self, gen):
                        self.gen = gen
                        self.done = 0
                        self.exhausted = gen is None

                    def pull(self, k=1):
                        for _ in range(k):
                            if self.exhausted:
                                return
                            if next(self.gen, "done") == "done":
                                self.exhausted = True
                            else:
                                self.done += 1

                    def ensure(self, n):
                        while not self.exhausted and self.done < n:
                            self.pull(1)

                    def drain(self):
                        while not self.exhausted:
                            self.pull(1)

                # units emitted per s-block m: 4 for m<4, else 3
                GCUM = [4, 8, 12, 16, 19, 22, 25, 28]

                def emit_u(h, inter):
                    """W^T produced directly: U^T[j, t] = sum_c E_P[c,j] *
                    (qh[t,c]/(t+1)) + sum_s A1[s,j] * maskinv-scoreT[s,t] —
                    both contractions have operands already j-/s-partitioned,
                    so the W transposes and their PSUM->SBUF copies vanish.
                    exp runs straight on the U^T psum (per-t scales were
                    folded into qhTs and the maskinvE strip constants); the
                    softmax denominator is recovered in emit_c from
                    ones-column matmuls against wtT."""
                    a1 = st[h][0]
                    g, p0 = h // 2, (h % 2) * 64
                    wtT = ap.tile([128, NB, S], BF16, tag="wtT", bufs=1, name="wtT")
                    Ets = {}
                    strips = {}
                    for half in range(2):
                        for P in (2 * half, 2 * half + 1):
                            inter.ensure(GCUM[min(2 * P + 3, NB - 1)])
                            ps_sc = ppm.tile([128, 384], F32, tag="cd", bufs=2,
                                             name="ps_sc")
                            nc.tensor.matmul(
                                ps_sc[:, 0:256],
                                khT[p0:p0 + 64, g, 2 * P * 128:(2 * P + 1) * 128],
                                qhT[p0:p0 + 64, g, 256 * P:256 * P + 256],
                                start=True, stop=True)
                            nc.tensor.matmul(
                                ps_sc[:, 256:384],
                                khT[p0:p0 + 64, g, (2 * P + 1) * 128:(2 * P + 2) * 128],
                                qhT[p0:p0 + 64, g, 256 * P + 128:256 * P + 256],
                                start=True, stop=True)
                            strip = sp.tile([128, 384], BF16, tag="scst", bufs=6,
                                            name="strip")
                            strips[P] = strip
                            nc.vector.tensor_tensor(strip[:], ps_sc[:],
                                                    miE[:, P, :], ALU.mult)
                            inter.pull(1)
                            if P < 3:
                                psEs = [ppm.tile([128, 512], F32, tag="pse", bufs=2,
                                                 name="ps_e") for _ in range(2)]
                                for c2 in range(2):
                                    for sb in (2 * P, 2 * P + 1):
                                        nc.tensor.matmul(
                                            psEs[c2][p0:p0 + 64, :],
                                            khn[:, g, sb, p0:p0 + 64],
                                            a1[:, sb, c2 * 512:(c2 + 1) * 512],
                                            start=(sb == 2 * P), stop=(sb == 2 * P + 1))
                                Enext = sp.tile([128, S], BF16, tag="E", bufs=4,
                                                name="E")
                                Eprev = Ets.get(P)
                                for c2 in range(2):
                                    if Eprev is None:
                                        nc.vector.tensor_copy(
                                            Enext[p0:p0 + 64, c2 * 512:(c2 + 1) * 512],
                                            psEs[c2][p0:p0 + 64, :])
                                    else:
                                        nc.vector.tensor_tensor(
                                            Enext[p0:p0 + 64, c2 * 512:(c2 + 1) * 512],
                                            Eprev[p0:p0 + 64, c2 * 512:(c2 + 1) * 512],
                                            psEs[c2][p0:p0 + 64, :], ALU.add)
                                Ets[P + 1] = Enext
                                inter.pull(1)
                            inter.pull(1)
                        # U^T for this half's 512 t-columns, one psum per
                        # j-block with clean 128-wide accumulation regions
                        t0 = 512 * half
                        for jb in range(NB):
                            ps = ppm.tile([128, 512], F32,
                                          tag=("mm" if jb % 2 == 0 else "pse"),
                                          bufs=2, name="ps_u")
                            for P in (2 * half, 2 * half + 1):
                                c0 = (P % 2) * 256
                                strip = strips[P]
                                Et = Ets.get(P)
                                for ti in range(2):
                                    r0 = c0 + ti * 128
                                    first = True
                                    if Et is not None:
                                        nc.tensor.matmul(
                                            ps[:, r0:r0 + 128],
                                            Et[p0:p0 + 64, jb * 128:(jb + 1) * 128],
                                            qhTs[p0:p0 + 64, g, 256 * P + ti * 128:
                                                 256 * P + (ti + 1) * 128],
                                            start=True, stop=False)
                                        first = False
                                    nc.tensor.matmul(
                                        ps[:, r0:r0 + 128],
                                        a1[:, 2 * P, jb * 128:(jb + 1) * 128],
                                        strip[:, ti * 128:(ti + 1) * 128],
                                        start=first, stop=(ti == 0))
                                    if ti == 1:
                                        nc.tensor.matmul(
                                            ps[:, r0:r0 + 128],
                                            a1[:, 2 * P + 1, jb * 128:(jb + 1) * 128],
                                            strip[:, 256:384],
                                            start=False, stop=True)
                            nc.scalar.activation(
                                wtT[:, jb, t0:t0 + 512], ps[:], ACTF.Exp)
                            if jb % 3 == 2:
                                inter.pull(1)
                    inter.drain()
                    if DEBUG and h == 0:
                        nc.sync.dma_start(
                            out=dbg["wtT"].rearrange("p (a b) -> p a b", a=NB), in_=wtT[:])
                    st[h].append(wtT)

                oNs = {}

                def emit_c(h, inter=None):
                    """out[t-block i] = gsc[t] * (W-blk_i @ C^(i) +
                    tril(W_i A1_i^T) @ vh_i) with the running accumulator
                    C^(i)[j, d] = sum_{s < 128i} A1[s, j] vh[s, d] carried in
                    bf16 (one DVE add per block).  gsc = 1/(den*(t+1)) as a
                    per-partition ACT scale (W was left unnormalized).  Heads
                    h, h+1 share one oN tile (free-axis halves) so a single
                    [128,128] PE transpose per t-block yields the stacked
                    [d, t] layout and the output projection contracts K=128
                    per head-pair."""
                    def pull(k):
                        if inter is not None:
                            inter.pull(k)
                    a1, sqT, wtT, gsc = st.pop(h)
                    d0h = h * 64
                    if h % 2 == 0:
                        oNs[h // 2] = sp.tile([128, NB, 128], BF16, tag="oN",
                                              bufs=3, name="oN")
                    oN = oNs[h // 2]
                    d0 = (h % 2) * 64

                    # a1T strips are produced one block ahead of the S2-diag
                    # matmuls that consume them; the PSUM->SBUF copies
                    # alternate between ACT and DVE to balance engine load
                    a1Ts = {}

                    def emit_a1t(m):
                        a1T = sp.tile([128, NB, 128], BF16, tag="a1T", bufs=8,
                                      name="a1T")
                        a1Ts[m] = a1T
                        tps = ppt.tile([128, S], BF16, tag="tp", name="tps2")
                        for k in range(NB):
                            nc.tensor.transpose(
                                tps[:, k * 128:(k + 1) * 128],
                                a1[:, m, k * 128:(k + 1) * 128], ident[:])
                        tv = tps[:].rearrange("p (a b) -> p a b", a=NB)
                        if m % 2 == 0:
                            nc.scalar.activation(a1T[:], tv, ACTF.Copy)
                        else:
                            nc.vector.tensor_copy(a1T[:], tv)

                    cprev = None
                    emit_a1t(0)
                    for i in range(NB):
                        if i + 1 < NB:
                            emit_a1t(i + 1)
                        a1T = a1Ts.pop(i)
                        # S2-diagonal block: S2dT[s, t] = sum_j A1[s,j] W[t,j]
                        psd = ppm.tile([128, 128], F32, tag="cd", bufs=2,
                                       name="ps_s2d")
                        for k in range(NB):
                            nc.tensor.matmul(
                                psd[:], a1T[:, k, :],
                                wtT[:, k, i * 128:(i + 1) * 128],
                                start=(k == 0), stop=(k == NB - 1))
                        s2dT = sp.tile([128, 128], BF16, tag="s2d", bufs=6,
                                       name="s2dT")
                        nc.vector.tensor_tensor(s2dT[:], psd[:],
                                                mask[:, 0, 0:128], ALU.mult)
                        # out-block i: prefix part via C (no dep on the mask),
                        # then the C update, then the diag part — keeps PE fed
                        # while the DVE mask / C-add drain
                        pso = ppm.tile([128, 66], F32, tag="cd", bufs=2,
                                       name="ps_o")
                        if i > 0:
                            for k in range(NB):
                                nc.tensor.matmul(
                                    pso[:, 0:64], wtT[:, k, i * 128:(i + 1) * 128],
                                    cprev[:, k, :],
                                    start=(k == 0), stop=False)
                        psc = None
                        if i + 1 < NB:
                            psc = ppm.tile([128, 512], F32, tag="mm",
                                           name="ps_cu")
                            for k in range(NB):
                                nc.tensor.matmul(
                                    psc[:, k * 64:(k + 1) * 64],
                                    a1[:, i, k * 128:(k + 1) * 128],
                                    vh[:, i, d0h:d0h + 64],
                                    start=True, stop=True)
                        nc.tensor.matmul(pso[:, 0:64], s2dT[:],
                                         vh[:, i, d0h:d0h + 64],
                                         start=(i == 0), stop=True)
                        # softmax denominator for this t-block: ones-column
                        # contraction over all j (after the out-group so the
                        # [0:64) accumulation is never interleaved)
                        for k in range(NB):
                            nc.tensor.matmul(
                                pso[:, 64:65], wtT[:, k, i * 128:(i + 1) * 128],
                                onescol[:, 0:1],
                                start=(k == 0), stop=(k == NB - 1))
                        rden = sp.tile([128, 1], F32, tag="rden", bufs=4,
                                       name="rden")
                        nc.vector.reciprocal(rden[:], pso[:, 64:65])
                        gsc = sp.tile([128, 1], F32, tag="gsc", bufs=4,
                                      name="gsc")
                        nc.vector.tensor_tensor(gsc[:], rden[:],
                                                invidx[:, i:i + 1], ALU.mult)
                        nc.scalar.activation(oN[:, i, d0:d0 + 64], pso[:, 0:64],
                                             ACTF.Copy, scale=gsc[:, 0:1])
                        # C update: C^(i+1) = C^(i) + A1_i^T @ vh_i
                        if psc is not None:
                            cnew = sp.tile([128, NB, 64], BF16, tag="C",
                                           bufs=6, name="C")
                            pv = psc[:].rearrange("p (a b) -> p a b", a=NB)
                            if cprev is None:
                                nc.vector.tensor_copy(cnew[:], pv)
                            else:
                                nc.vector.tensor_tensor(cnew[:], cprev[:], pv,
                                                        ALU.add)
                            cprev = cnew
                        pull(1)
                        if h == HG - 1:
                            # pipeline the pair-1 oT transpose and the output
                            # projection for t-block i into the last head's
                            # C-phase instead of a serial tail
                            tpo = ppm.tile([128, 128], BF16, tag="mm",
                                           name="tpo")
                            nc.tensor.transpose(tpo[:], oN[:, i, :], ident[:])
                            nc.scalar.activation(
                                oT[:, h // 2, i * 128:(i + 1) * 128], tpo[:],
                                ACTF.Copy)
                            emit_final_tile(i)
                    if h % 2 == 1 and h != HG - 1:
                        oNp = oNs.pop(h // 2)
                        tps = ppm.tile([128, S], BF16, tag="pse", bufs=2,
                                       name="tpo")
                        for i in range(NB):
                            nc.tensor.transpose(
                                tps[:, i * 128:(i + 1) * 128], oNp[:, i, :],
                                ident[:])
                        nc.scalar.activation(
                            oT[:, h // 2, :],
                            tps[:].rearrange("p (a b) -> p a b", a=NB),
                            ACTF.Copy)
                    if h == HG - 1:
                        oNs.pop(h // 2)

                def emit_final_tile(i):
                    # out[t-block i, :] = sum_g oT_g^T wc_g (all scales already
                    # folded into oT).  Runs inside emit_c(3) where the gen is
                    # done, so the a1ps ring is free — avoids contending with
                    # the C-update psums on the mm ring.
                    for c in range(2):
                        ps = ppm.tile([128, 512], F32, tag="a1ps", bufs=3,
                                      name="ps_fin")
                        for g2 in range(2):
                            nc.tensor.matmul(
                                ps[:], oT[:, g2, i * 128:(i + 1) * 128],
                                wct[:, g2, c * 512:(c + 1) * 512],
                                start=(g2 == 0), stop=(g2 == 1))
                        ot = sp.tile([128, 512], F32, tag="ot", bufs=8, name="ot")
                        if (i + c) % 2 == 0:
                            nc.scalar.activation(ot[:], ps[:], ACTF.Copy)
                        else:
                            nc.vector.tensor_copy(ot[:], ps[:])
                        nc.sync.dma_start(
                            out=out_d[i * 128:(i + 1) * 128, c * 512:(c + 1) * 512],
                            in_=ot[:])

                # vh[s, d] = sum_c vT[c, s] wv[c, d] + wv_b[d], interleaved
                # with head 0's A1/SqT so PE has work while vT streams in
                # kb-outer in two 4-block waves (two [128,512] psum tiles hold
                # m-pairs side by side) so PE consumes vT chunks as the DMA
                # stream lands them
                gen0 = GenState(gen_a1_sq(0))
                for w in range(2):
                    pss = [ppm.tile([128, 512], F32, tag="cd", bufs=2,
                                    name="ps_vh") for _ in range(2)]
                    for kb in range(NB):
                        for mi in range(4):
                            m = 4 * w + mi
                            nc.tensor.matmul(
                                pss[mi // 2][:, (mi % 2) * 256:(mi % 2 + 1) * 256],
                                vTt[:, kb, m * 128:(m + 1) * 128], wvt[:, kb, :],
                                start=(kb == 0), stop=False)
                        gen0.pull(1)
                    for mi in range(4):
                        m = 4 * w + mi
                        nc.tensor.matmul(
                            pss[mi // 2][:, (mi % 2) * 256:(mi % 2 + 1) * 256],
                            ones1[:], wvb[:], start=False, stop=True)
                        nc.scalar.activation(
                            vh[:, m, :],
                            pss[mi // 2][:, (mi % 2) * 256:(mi % 2 + 1) * 256],
                            ACTF.Copy)
                        gen0.pull(1)
                if DEBUG:
                    nc.sync.dma_start(out=dbg["vh"].rearrange("p (a b) -> p a b", a=NB),
                                      in_=vh[:])
                pend = gen0
                for h in range(HG):
                    emit_u(h, inter=pend)
                    pend = GenState(gen_a1_sq(h + 1) if h + 1 < HG else None)
                    emit_c(h, inter=pend)

            if DEBUG:
                nc.sync.dma_start(
                    out=dbg["oT"].rearrange("p (a b) -> p a b", a=HG), in_=oT[:])

            vp_cm.__exit__(None, None, None)

    nc.finalize()
    return nc


_CACHE = {}


def _get_program():
    if "nc" not in _CACHE:
        _CACHE["nc"] = _build_program()
    return _CACHE["nc"]


def _consts():
    if "consts" not in _CACHE:
        p_ = np.arange(128, dtype=np.float32)[:, None]
        c_ = np.arange(512, dtype=np.float32)[None, :]
        mask4 = np.stack(
            [(p_ + 128.0 * r <= c_) for r in range(4)]).astype(NPBF)
        ident = np.eye(128, dtype=np.float32).astype(NPBF)
        blk = np.arange(NB, dtype=np.float32)[None, :]
        invidx = (1.0 / (blk * 128.0 + p_ + 1.0)).astype(np.float32)
        ones1 = np.ones((1, 128), NPBF)
        # maskinvE[P][p, c]: causal mask times 1/(t+1) for the two scoreT
        # strips of pair P (s-block 2P vs t in pair; diag of 2P+1)
        mie = np.zeros((4, 128, 384), np.float32)
        for P in range(4):
            c = np.arange(256, dtype=np.float32)[None, :]
            mie[P, :, 0:256] = (p_ <= c) / (256.0 * P + c + 1.0)
            c2 = np.arange(128, dtype=np.float32)[None, :]
            mie[P, :, 256:384] = (p_ <= c2) / (256.0 * P + 128.0 + c2 + 1.0)
        t_ = np.arange(S, dtype=np.float32)[None, :]
        invtbc = np.broadcast_to(1.0 / (t_ + 1.0), (128, S)).astype(np.float32)
        invtbc = np.ascontiguousarray(invtbc)
        _CACHE["consts"] = (mask4, ident, invidx, ones1, mie, invtbc)
    return _CACHE["consts"]


PROFILE = False
LAST_RESULTS = None


def kernel(v, k, q, p, wq_k, wq_b, wk_k, wk_b, wv_k, wv_b, wc_k, wc_b):
    global LAST_RESULTS
    nc = _get_program()
    mask4, ident, invidx, ones1, mie, invtbc = _consts()

    qT = [np.ascontiguousarray(q[b].T).astype(NPBF) for b in range(B)]
    kT = [np.ascontiguousarray(k[b].T).astype(NPBF) for b in range(B)]
    vT = [np.ascontiguousarray(v[b].T).astype(NPBF) for b in range(B)]
    pT = [np.ascontiguousarray(p[b].T).astype(NPBF) for b in range(B)]
    wqc = wq_k.astype(NPBF)
    wkc = wk_k.astype(NPBF)
    wvc = wv_k.astype(NPBF)
    wcc = wc_k.astype(NPBF)

    in_maps = []
    for c in range(8):
        b, hg = c // 4, c % 4
        c0 = hg * DL
        wqb = np.ascontiguousarray(
            (wq_b[c0:c0 + DL].reshape(2, 128).T * NORM_D).astype(np.float32))
        wkb = np.ascontiguousarray(wk_b[c0:c0 + DL].reshape(2, 128).T.astype(np.float32))
        in_maps.append({
            "qT": qT[b], "kT": kT[b], "vT": vT[b],
            "pT": np.ascontiguousarray(pT[b][c0:c0 + DL]),
            "wq": np.ascontiguousarray(wqc[:, c0:c0 + DL]),
            "wk": np.ascontiguousarray(wkc[:, c0:c0 + DL]),
            "wv": np.ascontiguousarray(wvc[:, c0:c0 + DL]),
            "wc": np.ascontiguousarray(wcc[c0:c0 + DL, :]),
            "wqb": wqb, "wkb": wkb,
            "wvb": np.ascontiguousarray(wv_b[c0:c0 + DL].reshape(1, DL).astype(NPBF)),
            "ones1": ones1, "mask4": mask4, "ident": ident, "invidx": invidx,
            "maskinvE": mie, "invtbc": invtbc,
        })

    res = run_bass_kernel_spmd(
        nc, in_maps, core_ids=list(range(8)), trace=PROFILE)
    LAST_RESULTS = res

    out = np.zeros((B, S, DM), np.float32)
    for c in range(8):
        out[c // 4] += res.results[c]["out"]
    out += wc_b[None, None, :].astype(np.float32)
    return out



# revision 77
# speedup vs baseline: 1.0003x; 1.0003x over previous
"""Trainium2 Bass kernel for nn_MultiHeadAttention_75548474736720.

Linear-attention-style multi-head attention with causal prefix sums:
  qh/kh/vh = projections, ph = split_heads(p)
  A1 = elu(qh ph^T) + 1                       [t,s] per (b,h)
  U  = (tril(qh kh^T)/idx) @ A1 ; W = softmax(U)
  out[t] = (1/(t+1)) sum_{s<=t} (W[t]·A1[s]) vh[s] ; reshape @ wc + b

Sharding: 8 cores = (batch b in 0..1) x (head-group hg in 0..3, 4 heads each).
Each core computes its 4 heads end-to-end (wq/wk/wv column-sliced, wc
row-sliced) and returns a partial [S, Dm] output; host sums partials per batch.

All matmuls run in bf16 (f32 PSUM accumulation).  Key algebraic tricks:
  - exp without max-subtraction (scaled U bounded ~|19| for this family)
  - first prefix sum via a running accumulator E_P[c,j] = sum_{s<256P}
    kh[s,c] A1[s,j] per head: U for a 256-row pair = qh @ E_P plus two
    masked scoreT strips, instead of the O(S*S) masked-score sweep
  - W^T produced DIRECTLY: U^T[j,t] = E^T @ (qh/(t+1)) + A1^T @ strips has
    both contractions j-/s-partitioned already, so no W transposes or
    PSUM->SBUF copies; the per-t softmax scale is folded into a pre-scaled
    qh copy (qhTs) and mask*1/(t+1) strip constants (maskinvE)
  - softmax denominator recovered per t-block by N=1 ones-column matmuls
    against W^T (emitted after the out-group closes: interleaving a
    start=True sub-range group inside an open psum group corrupts it)
  - second prefix sum via a running accumulator C[j,d] = sum_{s<t0} A1[s,j]
    vh[s,d] per head: out-block i = W-block @ C + tril(W A1_i^T) @ vh_i,
    which is O(S*S*D) instead of O(S*S*S) for the explicit S2 matrix
"""

import sys

sys.path.insert(0, "/opt/trn_rl_repo")

import ml_dtypes
import numpy as np

import concourse.bass as bass  # noqa: F401  (registers AP machinery)
import concourse.mybir as mybir
from concourse import bacc
from concourse.tile import TileContext
from concourse.bass_utils import run_bass_kernel_spmd

F32 = mybir.dt.float32
BF16 = mybir.dt.bfloat16
ACTF = mybir.ActivationFunctionType
ALU = mybir.AluOpType
NPBF = ml_dtypes.bfloat16

B, S, DM, H = 2, 1024, 1024, 16
D = DM // H            # 64, head dim
HG = 4                 # heads per core
DL = HG * D            # 256, local dm slice
NB = S // 128          # 8 s-blocks
NORM_D = 0.125         # 1/sqrt(D)

# compact SqT layout: per s-block m, columns stored from t = 512*(m//4)
SQBASE = [0, 1024, 2048, 3072, 4096, 4608, 5120, 5632]  # total 6144

DEBUG = False


def _sq_off(m, t0):
    return SQBASE[m] + t0 - 512 * (m // 4)


def _build_program():
    nc = bacc.Bacc(None, target_bir_lowering=False)

    qT_in = nc.declare_dram_parameter("qT", [DM, S], BF16, isOutput=False)
    kT_in = nc.declare_dram_parameter("kT", [DM, S], BF16, isOutput=False)
    vT_in = nc.declare_dram_parameter("vT", [DM, S], BF16, isOutput=False)
    pT_in = nc.declare_dram_parameter("pT", [DL, S], BF16, isOutput=False)
    wq_in = nc.declare_dram_parameter("wq", [DM, DL], BF16, isOutput=False)
    wk_in = nc.declare_dram_parameter("wk", [DM, DL], BF16, isOutput=False)
    wv_in = nc.declare_dram_parameter("wv", [DM, DL], BF16, isOutput=False)
    wc_in = nc.declare_dram_parameter("wc", [DL, S], BF16, isOutput=False)
    wqb_in = nc.declare_dram_parameter("wqb", [128, 2], F32, isOutput=False)
    wkb_in = nc.declare_dram_parameter("wkb", [128, 2], F32, isOutput=False)
    wvb_in = nc.declare_dram_parameter("wvb", [1, DL], BF16, isOutput=False)
    ones_in = nc.declare_dram_parameter("ones1", [1, 128], BF16, isOutput=False)
    mask_in = nc.declare_dram_parameter("mask4", [4, 128, 512], BF16, isOutput=False)
    ident_in = nc.declare_dram_parameter("ident", [128, 128], BF16, isOutput=False)
    inv_in = nc.declare_dram_parameter("invidx", [128, NB], F32, isOutput=False)
    mie_in = nc.declare_dram_parameter("maskinvE", [4, 128, 384], F32, isOutput=False)
    invb_in = nc.declare_dram_parameter("invtbc", [128, S], F32, isOutput=False)
    out_d = nc.declare_dram_parameter("out", [S, DM], F32, isOutput=True)
    dbg = {}
    if DEBUG:
        dbg["qhT"] = nc.declare_dram_parameter("d_qhT", [128, 2 * S], F32, isOutput=True)
        dbg["vh"] = nc.declare_dram_parameter("d_vh", [128, NB * DL], F32, isOutput=True)
        dbg["a1"] = nc.declare_dram_parameter("d_a1", [128, NB * S], F32, isOutput=True)
        dbg["sqT"] = nc.declare_dram_parameter("d_sqT", [128, 6144], F32, isOutput=True)
        dbg["wtT"] = nc.declare_dram_parameter("d_wtT", [128, NB * S], F32, isOutput=True)
        dbg["oT"] = nc.declare_dram_parameter("d_oT", [64, HG * S], F32, isOutput=True)
        dbg["den"] = nc.declare_dram_parameter("d_den", [128, NB], F32, isOutput=True)

    with TileContext(nc) as tc:
        with tc.tile_pool(name="persist", bufs=1) as cp, \
             tc.tile_pool(name="ppm", bufs=2, space="PSUM") as ppm, \
             tc.tile_pool(name="ppt", bufs=1, space="PSUM") as ppt:

            mask = cp.tile([128, 4, 512], BF16)
            ident = cp.tile([128, 128], BF16)
            invidx = cp.tile([128, NB], F32)
            wqb = cp.tile([128, 2], F32)
            wkb = cp.tile([128, 2], F32)
            wvb = cp.tile([1, DL], BF16)
            ones1 = cp.tile([1, 128], BF16)
            pTt = cp.tile([128, 2, S], BF16)
            qhT = cp.tile([128, 2, S], BF16)
            khT = cp.tile([128, 2, S], BF16)
            vh = cp.tile([128, NB, DL], BF16)
            oT = cp.tile([128, 2, S], BF16)
            # wc stored per head-pair: wct[:, g, :] = wc rows [g*128:(g+1)*128];
            # loaded up front so the output projection never waits on DMA
            wct = cp.tile([128, 2, S], BF16)

            # ---------------- projections ----------------
            # DMA issue on SP costs ~0.5us per descriptor, so the inputs the
            # first matmuls need go first, split 4-ways for queue parallelism;
            # constants (masks, wc, p) follow.  The v projection runs inside
            # the attention phase (interleaved with head 0's A1/SqT) so its
            # tiles live in a separate pool that outlives the q/k one.
            vp_cm = tc.tile_pool(name="vproj", bufs=1)
            vp = vp_cm.__enter__()
            wvt = vp.tile([128, NB, DL], BF16)
            vTt = vp.tile([128, NB, S], BF16)
            with tc.tile_pool(name="proj", bufs=1) as jp:
                wqt = jp.tile([128, NB, DL], BF16)
                wkt = jp.tile([128, NB, DL], BF16)
                qTt = jp.tile([128, NB, S], BF16)
                kTt = jp.tile([128, NB, S], BF16)
                def ldw(wt_, wsrc, kb, n=2):
                    nc.sync.dma_start(
                        out=wt_[:, kb:kb + n, :],
                        in_=wsrc[kb * 128:(kb + n) * 128, :].rearrange(
                            "(a p) d -> p a d", p=128))

                def ldx(xt_, xsrc, kb, n=2):
                    nc.sync.dma_start(
                        out=xt_[:, kb:kb + n, :],
                        in_=xsrc[kb * 128:(kb + n) * 128, :].rearrange(
                            "(a p) t -> p a t", p=128))

                # transfers are serialized on the DMA path, so issue in the
                # exact order the kb-outer projection matmuls consume them,
                # single-block input chunks so PE streams behind the DMA
                ldw(wqt, wq_in, 0, 4)
                ldx(qTt, qT_in, 0, 1); ldx(qTt, qT_in, 1, 1)
                ldw(wqt, wq_in, 4, 4)
                nc.sync.dma_start(out=wqb[:], in_=wqb_in[:])
                for kb in range(2, NB):
                    ldx(qTt, qT_in, kb, 1)
                ldw(wkt, wk_in, 0, 4); ldw(wkt, wk_in, 4, 4)
                nc.sync.dma_start(out=wkb[:], in_=wkb_in[:])
                for kb in range(NB):
                    ldx(kTt, kT_in, kb, 1)
                nc.sync.dma_start(
                    out=pTt[:], in_=pT_in.rearrange("(g p) t -> p g t", p=128))
                nc.sync.dma_start(out=invidx[:], in_=inv_in[:])
                nc.sync.dma_start(
                    out=mask[:], in_=mask_in.rearrange("r p c -> p r c"))
                nc.sync.dma_start(out=ident[:], in_=ident_in[:])
                ldw(wvt, wv_in, 0, 4); ldw(wvt, wv_in, 4, 4)
                for kb in range(0, NB, 2):
                    ldx(vTt, vT_in, kb)
                nc.sync.dma_start(out=wvb[:], in_=wvb_in[:])
                nc.sync.dma_start(out=ones1[:], in_=ones_in[:])
                nc.sync.dma_start(
                    out=miE[:], in_=mie_in.rearrange("r p c -> p r c"))
                nc.sync.dma_start(out=invtbc[:], in_=invb_in[:])
                nc.sync.dma_start(
                    out=wct[:], in_=wc_in.rearrange("(a p) t -> p a t", p=128))

                # qhT[dm, t] = sum_c wq[c, dm] qT[c, t]  (+bias, * 1/sqrt(D))
                # kb-outer accumulation so PE consumes input chunks as the
                # (serialized) DMA stream lands them instead of waiting for
                # the full operand
                for wt_, xt_, dst, bias_t, scale in (
                    (wqt, qTt, qhT, wqb, NORM_D),
                    (wkt, kTt, khT, wkb, 1.0),
                ):
                    for g in range(2):
                        pss = [ppm.tile([128, 512], F32, tag="mm", name="ps_proj")
                               for _ in range(2)]
                        for kb in range(NB):
                            for n in range(2):
                                nc.tensor.matmul(
                                    pss[n][:], wt_[:, kb, g * 128:(g + 1) * 128],
                                    xt_[:, kb, n * 512:(n + 1) * 512],
                                    start=(kb == 0), stop=(kb == NB - 1))
                        for n in range(2):
                            nc.scalar.activation(
                                dst[:, g, n * 512:(n + 1) * 512], pss[n][:],
                                ACTF.Identity, bias=bias_t[:, g:g + 1], scale=scale)

                if DEBUG:
                    nc.sync.dma_start(out=dbg["qhT"].rearrange("p (a b) -> p a b", a=2),
                                      in_=qhT[:])

            # ---------------- attention (4 heads) ----------------
            # Pair-level software pipeline: A1/SqT for head h+1 are emitted
            # between U(h) and S2(h) so the in-order PE stream always has
            # independent matmuls to run while elementwise chains drain.
            with tc.tile_pool(name="attn", bufs=2) as ap, \
                 tc.tile_pool(name="scr", bufs=2) as sp:
                st = {}

                def gen_a1_sq(h):
                    """Generator: yields after each matmul unit so A1/SqT of
                    head h can be interleaved into head h-1's S2 phase (keeps
                    the in-order PE queue fed while elementwise chains drain).

                    A1 = elu(x)+1 = min(exp(x), 1) + relu(x); exp is safe
                    unclamped (|x| <= ~8 here).  The PSUM tile is released
                    after just two reads (exp on ACT, relu on DVE, which run
                    concurrently); min and the final add run on idle GPSIMD
                    from SBUF so the a1ps ring turns over fast."""
                    g, p0 = h // 2, (h % 2) * 64
                    a1 = ap.tile([128, NB, S], BF16, tag="a1", name="a1")
                    sqT = ap.tile([128, 6144], BF16, tag="sq", name="sqT")
                    st[h] = [a1, sqT]

                    def a1_unit(m, c):
                        ps = ppm.tile([128, 512], F32, tag="a1ps", bufs=3,
                                      name="ps_a1")
                        nc.tensor.matmul(
                            ps[:], qhT[p0:p0 + 64, g, m * 128:(m + 1) * 128],
                            pTt[p0:p0 + 64, g, c * 512:(c + 1) * 512],
                            start=True, stop=True)
                        e = sp.tile([128, 512], F32, tag="e", bufs=8, name="e")
                        nc.scalar.activation(e[:], ps[:], ACTF.Exp)
                        e1 = sp.tile([128, 512], F32, tag="e1", bufs=8, name="e1")
                        nc.gpsimd.tensor_scalar_min(e1[:], e[:], 1.0)
                        nc.vector.scalar_tensor_tensor(
                            a1[:, m, c * 512:(c + 1) * 512], ps[:], 0.0, e1[:],
                            ALU.max, ALU.add)

                    def sq_unit(m, n):
                        ps = ppm.tile([128, 512], F32, tag="mm", name="ps_sq")
                        nc.tensor.matmul(
                            ps[:], khT[p0:p0 + 64, g, m * 128:(m + 1) * 128],
                            qhT[p0:p0 + 64, g, n * 512:(n + 1) * 512],
                            start=True, stop=True)
                        dst = sqT[:, _sq_off(m, n * 512):_sq_off(m, n * 512) + 512]
                        if n == m // 4:
                            nc.vector.tensor_tensor(dst, ps[:], mask[:, m % 4, :], ALU.mult)
                        else:
                            nc.scalar.activation(dst, ps[:], ACTF.Copy)

                    # interleave A1 and sq units so consecutive A1 psum tiles
                    # (whose consumer chain is long) are spaced ~2 units apart
                    for m in range(NB):
                        a1_unit(m, 0)
                        yield
                        sq_unit(m, 0 if m < 4 else 1)
                        yield
                        a1_unit(m, 1)
                        yield
                        if m < 4:
                            sq_unit(m, 1)
                            yield
                    if DEBUG and h == 0:
                        nc.sync.dma_start(
                            out=dbg["a1"].rearrange("p (a b) -> p a b", a=NB), in_=a1[:])
                        nc.sync.dma_start(out=dbg["sqT"][:, :], in_=sqT[:])

                class GenState:
                    """Paces a gen_a1_sq generator across phases; tracks how
                    many units have been emitted so consumers can require
                    progress (U-block i needs all units for s-blocks <= i)."""
                    def __init__(self, gen):
                        self.gen = gen
                        self.done = 0
                        self.exhausted = gen is None

                    def pull(self, k=1):
                        for _ in range(k):
                            if self.exhausted:
                                return
                            if next(self.gen, "done") == "done":
                                self.exhausted = True
                            else:
                                self.done += 1

                    def ensure(self, n):
                        while not self.exhausted and self.done < n:
                            self.pull(1)

                    def drain(self):
                        while not self.exhausted:
                            self.pull(1)

                # units emitted per s-block m: 4 for m<4, else 3
                GCUM = [4, 8, 12, 16, 19, 22, 25, 28]

                def emit_u(h, inter):
                    """W^T produced directly: U^T[j, t] = sum_c E_P[c,j] *
                    (qh[t,c]/(t+1)) + sum_s A1[s,j] * maskinv-scoreT[s,t] —
                    both contractions have operands already j-/s-partitioned,
                    so the W transposes and their PSUM->SBUF copies vanish.
                    exp runs straight on the U^T psum (per-t scales were
                    folded into qhTs and the maskinvE strip constants); the
                    softmax denominator is recovered in emit_c from
                    ones-column matmuls against wtT."""
                    a1 = st[h][0]
                    g, p0 = h // 2, (h % 2) * 64
                    wtT = ap.tile([128, NB, S], BF16, tag="wtT", bufs=1, name="wtT")
                    Ets = {}
                    strips = {}
                    for half in range(2):
                        for P in (2 * half, 2 * half + 1):
                            inter.ensure(GCUM[min(2 * P + 5, NB - 1)])
                            ps_sc = ppm.tile([128, 384], F32, tag="cd", bufs=2,
                                             name="ps_sc")
                            nc.tensor.matmul(
                                ps_sc[:, 0:256],
                                khT[p0:p0 + 64, g, 2 * P * 128:(2 * P + 1) * 128],
                                qhT[p0:p0 + 64, g, 256 * P:256 * P + 256],
                                start=True, stop=True)
                            nc.tensor.matmul(
                                ps_sc[:, 256:384],
                                khT[p0:p0 + 64, g, (2 * P + 1) * 128:(2 * P + 2) * 128],
                                qhT[p0:p0 + 64, g, 256 * P + 128:256 * P + 256],
                                start=True, stop=True)
                            strip = sp.tile([128, 384], BF16, tag="scst", bufs=6,
                                            name="strip")
                            strips[P] = strip
                            nc.vector.tensor_tensor(strip[:], ps_sc[:],
                                                    miE[:, P, :], ALU.mult)
                            inter.pull(1)
                            if P < 3:
                                psEs = [ppm.tile([128, 512], F32, tag="pse", bufs=2,
                                                 name="ps_e") for _ in range(2)]
                                for c2 in range(2):
                                    for sb in (2 * P, 2 * P + 1):
                                        nc.tensor.matmul(
                                            psEs[c2][p0:p0 + 64, :],
                                            khn[:, g, sb, p0:p0 + 64],
                                            a1[:, sb, c2 * 512:(c2 + 1) * 512],
                                            start=(sb == 2 * P), stop=(sb == 2 * P + 1))
                                Enext = sp.tile([128, S], BF16, tag="E", bufs=4,
                                                name="E")
                                Eprev = Ets.get(P)
                                for c2 in range(2):
                                    if Eprev is None:
                                        nc.vector.tensor_copy(
                                            Enext[p0:p0 + 64, c2 * 512:(c2 + 1) * 512],
                                            psEs[c2][p0:p0 + 64, :])
                                    else:
                                        nc.vector.tensor_tensor(
                                            Enext[p0:p0 + 64, c2 * 512:(c2 + 1) * 512],
                                            Eprev[p0:p0 + 64, c2 * 512:(c2 + 1) * 512],
                                            psEs[c2][p0:p0 + 64, :], ALU.add)
                                Ets[P + 1] = Enext
                                inter.pull(1)
                            inter.pull(1)
                        # U^T for this half's 512 t-columns, one psum per
                        # j-block with clean 128-wide accumulation regions
                        t0 = 512 * half
                        for jb in range(NB):
                            ps = ppm.tile([128, 512], F32,
                                          tag=("mm" if jb % 2 == 0 else "pse"),
                                          bufs=2, name="ps_u")
                            for P in (2 * half, 2 * half + 1):
                                c0 = (P % 2) * 256
                                strip = strips[P]
                                Et = Ets.get(P)
                                for ti in range(2):
                                    r0 = c0 + ti * 128
                                    first = True
                                    if Et is not None:
                                        nc.tensor.matmul(
                                            ps[:, r0:r0 + 128],
                                            Et[p0:p0 + 64, jb * 128:(jb + 1) * 128],
                                            qhTs[p0:p0 + 64, g, 256 * P + ti * 128:
                                                 256 * P + (ti + 1) * 128],
                                            start=True, stop=False)
                                        first = False
                                    nc.tensor.matmul(
                                        ps[:, r0:r0 + 128],
                                        a1[:, 2 * P, jb * 128:(jb + 1) * 128],
                                        strip[:, ti * 128:(ti + 1) * 128],
                                        start=first, stop=(ti == 0))
                                    if ti == 1:
                                        nc.tensor.matmul(
                                            ps[:, r0:r0 + 128],
                                            a1[:, 2 * P + 1, jb * 128:(jb + 1) * 128],
                                            strip[:, 256:384],
                                            start=False, stop=True)
                            nc.scalar.activation(
                                wtT[:, jb, t0:t0 + 512], ps[:], ACTF.Exp)
                            if jb % 3 == 2:
                                inter.pull(1)
                    inter.drain()
                    if DEBUG and h == 0:
                        nc.sync.dma_start(
                            out=dbg["wtT"].rearrange("p (a b) -> p a b", a=NB), in_=wtT[:])
                    st[h].append(wtT)

                oNs = {}

                def emit_c(h, inter=None):
                    """out[t-block i] = gsc[t] * (W-blk_i @ C^(i) +
                    tril(W_i A1_i^T) @ vh_i) with the running accumulator
                    C^(i)[j, d] = sum_{s < 128i} A1[s, j] vh[s, d] carried in
                    bf16 (one DVE add per block).  gsc = 1/(den*(t+1)) as a
                    per-partition ACT scale (W was left unnormalized).  Heads
                    h, h+1 share one oN tile (free-axis halves) so a single
                    [128,128] PE transpose per t-block yields the stacked
                    [d, t] layout and the output projection contracts K=128
                    per head-pair."""
                    def pull(k):
                        if inter is not None:
                            inter.pull(k)
                    a1, sqT, wtT, gsc = st.pop(h)
                    d0h = h * 64
                    if h % 2 == 0:
                        oNs[h // 2] = sp.tile([128, NB, 128], BF16, tag="oN",
                                              bufs=3, name="oN")
                    oN = oNs[h // 2]
                    d0 = (h % 2) * 64

                    # a1T strips are produced one block ahead of the S2-diag
                    # matmuls that consume them; the PSUM->SBUF copies
                    # alternate between ACT and DVE to balance engine load
                    a1Ts = {}

                    def emit_a1t(m):
                        a1T = sp.tile([128, NB, 128], BF16, tag="a1T", bufs=8,
                                      name="a1T")
                        a1Ts[m] = a1T
                        tps = ppt.tile([128, S], BF16, tag="tp", name="tps2")
                        for k in range(NB):
                            nc.tensor.transpose(
                                tps[:, k * 128:(k + 1) * 128],
                                a1[:, m, k * 128:(k + 1) * 128], ident[:])
                        tv = tps[:].rearrange("p (a b) -> p a b", a=NB)
                        if m % 2 == 0:
                            nc.scalar.activation(a1T[:], tv, ACTF.Copy)
                        else:
                            nc.vector.tensor_copy(a1T[:], tv)

                    cprev = None
                    emit_a1t(0)
                    for i in range(NB):
                        if i + 1 < NB:
                            emit_a1t(i + 1)
                        a1T = a1Ts.pop(i)
                        # S2-diagonal block: S2dT[s, t] = sum_j A1[s,j] W[t,j]
                        psd = ppm.tile([128, 128], F32, tag="cd", bufs=2,
                                       name="ps_s2d")
                        for k in range(NB):
                            nc.tensor.matmul(
                                psd[:], a1T[:, k, :],
                                wtT[:, k, i * 128:(i + 1) * 128],
                                start=(k == 0), stop=(k == NB - 1))
                        s2dT = sp.tile([128, 128], BF16, tag="s2d", bufs=6,
                                       name="s2dT")
                        nc.vector.tensor_tensor(s2dT[:], psd[:],
                                                mask[:, 0, 0:128], ALU.mult)
                        # out-block i: prefix part via C (no dep on the mask),
                        # then the C update, then the diag part — keeps PE fed
                        # while the DVE mask / C-add drain
                        pso = ppm.tile([128, 66], F32, tag="cd", bufs=2,
                                       name="ps_o")
                        if i > 0:
                            for k in range(NB):
                                nc.tensor.matmul(
                                    pso[:, 0:64], wtT[:, k, i * 128:(i + 1) * 128],
                                    cprev[:, k, :],
                                    start=(k == 0), stop=False)
                        psc = None
                        if i + 1 < NB:
                            psc = ppm.tile([128, 512], F32, tag="mm",
                                           name="ps_cu")
                            for k in range(NB):
                                nc.tensor.matmul(
                                    psc[:, k * 64:(k + 1) * 64],
                                    a1[:, i, k * 128:(k + 1) * 128],
                                    vh[:, i, d0h:d0h + 64],
                                    start=True, stop=True)
                        nc.tensor.matmul(pso[:, 0:64], s2dT[:],
                                         vh[:, i, d0h:d0h + 64],
                                         start=(i == 0), stop=True)
                        # softmax denominator for this t-block: ones-column
                        # contraction over all j (after the out-group so the
                        # [0:64) accumulation is never interleaved)
                        for k in range(NB):
                            nc.tensor.matmul(
                                pso[:, 64:65], wtT[:, k, i * 128:(i + 1) * 128],
                                onescol[:, 0:1],
                                start=(k == 0), stop=(k == NB - 1))
                        rden = sp.tile([128, 1], F32, tag="rden", bufs=4,
                                       name="rden")
                        nc.vector.reciprocal(rden[:], pso[:, 64:65])
                        gsc = sp.tile([128, 1], F32, tag="gsc", bufs=4,
                                      name="gsc")
                        nc.vector.tensor_tensor(gsc[:], rden[:],
                                                invidx[:, i:i + 1], ALU.mult)
                        nc.scalar.activation(oN[:, i, d0:d0 + 64], pso[:, 0:64],
                                             ACTF.Copy, scale=gsc[:, 0:1])
                        # C update: C^(i+1) = C^(i) + A1_i^T @ vh_i
                        if psc is not None:
                            cnew = sp.tile([128, NB, 64], BF16, tag="C",
                                           bufs=6, name="C")
                            pv = psc[:].rearrange("p (a b) -> p a b", a=NB)
                            if cprev is None:
                                nc.vector.tensor_copy(cnew[:], pv)
                            else:
                                nc.vector.tensor_tensor(cnew[:], cprev[:], pv,
                                                        ALU.add)
                            cprev = cnew
                        pull(1)
                        if h == HG - 1:
                            # pipeline the pair-1 oT transpose and the output
                            # projection for t-block i into the last head's
                            # C-phase instead of a serial tail
                            tpo = ppm.tile([128, 128], BF16, tag="mm",
                                           name="tpo")
                            nc.tensor.transpose(tpo[:], oN[:, i, :], ident[:])
                            nc.scalar.activation(
                                oT[:, h // 2, i * 128:(i + 1) * 128], tpo[:],
                                ACTF.Copy)
                            emit_final_tile(i)
                    if h % 2 == 1 and h != HG - 1:
                        oNp = oNs.pop(h // 2)
                        tps = ppm.tile([128, S], BF16, tag="pse", bufs=2,
                                       name="tpo")
                        for i in range(NB):
                            nc.tensor.transpose(
                                tps[:, i * 128:(i + 1) * 128], oNp[:, i, :],
                                ident[:])
                        nc.scalar.activation(
                            oT[:, h // 2, :],
                            tps[:].rearrange("p (a b) -> p a b", a=NB),
                            ACTF.Copy)
                    if h == HG - 1:
                        oNs.pop(h // 2)

                def emit_final_tile(i):
                    # out[t-block i, :] = sum_g oT_g^T wc_g (all scales already
                    # folded into oT).  Runs inside emit_c(3) where the gen is
                    # done, so the a1ps ring is free — avoids contending with
                    # the C-update psums on the mm ring.
                    for c in range(2):
                        ps = ppm.tile([128, 512], F32, tag="a1ps", bufs=3,
                                      name="ps_fin")
                        for g2 in range(2):
                            nc.tensor.matmul(
                                ps[:], oT[:, g2, i * 128:(i + 1) * 128],
                                wct[:, g2, c * 512:(c + 1) * 512],
                                start=(g2 == 0), stop=(g2 == 1))
                        ot = sp.tile([128, 512], F32, tag="ot", bufs=8, name="ot")
                        if (i + c) % 2 == 0:
                            nc.scalar.activation(ot[:], ps[:], ACTF.Copy)
                        else:
                            nc.vector.tensor_copy(ot[:], ps[:])
                        nc.sync.dma_start(
                            out=out_d[i * 128:(i + 1) * 128, c * 512:(c + 1) * 512],
                            in_=ot[:])

                # vh[s, d] = sum_c vT[c, s] wv[c, d] + wv_b[d], interleaved
                # with head 0's A1/SqT so PE has work while vT streams in
                # kb-outer in two 4-block waves (two [128,512] psum tiles hold
                # m-pairs side by side) so PE consumes vT chunks as the DMA
                # stream lands them
                gen0 = GenState(gen_a1_sq(0))
                for w in range(2):
                    pss = [ppm.tile([128, 512], F32, tag="cd", bufs=2,
                                    name="ps_vh") for _ in range(2)]
                    for kb in range(NB):
                        for mi in range(4):
                            m = 4 * w + mi
                            nc.tensor.matmul(
                                pss[mi // 2][:, (mi % 2) * 256:(mi % 2 + 1) * 256],
                                vTt[:, kb, m * 128:(m + 1) * 128], wvt[:, kb, :],
                                start=(kb == 0), stop=False)
                        gen0.pull(1)
                    for mi in range(4):
                        m = 4 * w + mi
                        nc.tensor.matmul(
                            pss[mi // 2][:, (mi % 2) * 256:(mi % 2 + 1) * 256],
                            ones1[:], wvb[:], start=False, stop=True)
                        nc.scalar.activation(
                            vh[:, m, :],
                            pss[mi // 2][:, (mi % 2) * 256:(mi % 2 + 1) * 256],
                            ACTF.Copy)
                        gen0.pull(1)
                if DEBUG:
                    nc.sync.dma_start(out=dbg["vh"].rearrange("p (a b) -> p a b", a=NB),
                                      in_=vh[:])
                pend = gen0
                for h in range(HG):
                    emit_u(h, inter=pend)
                    pend = GenState(gen_a1_sq(h + 1) if h + 1 < HG else None)
                    emit_c(h, inter=pend)

            if DEBUG:
                nc.sync.dma_start(
                    out=dbg["oT"].rearrange("p (a b) -> p a b", a=HG), in_=oT[:])

            vp_cm.__exit__(None, None, None)

    nc.finalize()
    return nc


_CACHE = {}


def _get_program():
    if "nc" not in _CACHE:
        _CACHE["nc"] = _build_program()
    return _CACHE["nc"]


def _consts():
    if "consts" not in _CACHE:
        p_ = np.arange(128, dtype=np.float32)[:, None]
        c_ = np.arange(512, dtype=np.float32)[None, :]
        mask4 = np.stack(
            [(p_ + 128.0 * r <= c_) for r in range(4)]).astype(NPBF)
        ident = np.eye(128, dtype=np.float32).astype(NPBF)
        blk = np.arange(NB, dtype=np.float32)[None, :]
        invidx = (1.0 / (blk * 128.0 + p_ + 1.0)).astype(np.float32)
        ones1 = np.ones((1, 128), NPBF)
        # maskinvE[P][p, c]: causal mask times 1/(t+1) for the two scoreT
        # strips of pair P (s-block 2P vs t in pair; diag of 2P+1)
        mie = np.zeros((4, 128, 384), np.float32)
        for P in range(4):
            c = np.arange(256, dtype=np.float32)[None, :]
            mie[P, :, 0:256] = (p_ <= c) / (256.0 * P + c + 1.0)
            c2 = np.arange(128, dtype=np.float32)[None, :]
            mie[P, :, 256:384] = (p_ <= c2) / (256.0 * P + 128.0 + c2 + 1.0)
        t_ = np.arange(S, dtype=np.float32)[None, :]
        invtbc = np.broadcast_to(1.0 / (t_ + 1.0), (128, S)).astype(np.float32)
        invtbc = np.ascontiguousarray(invtbc)
        _CACHE["consts"] = (mask4, ident, invidx, ones1, mie, invtbc)
    return _CACHE["consts"]


PROFILE = False
LAST_RESULTS = None


def kernel(v, k, q, p, wq_k, wq_b, wk_k, wk_b, wv_k, wv_b, wc_k, wc_b):
    global LAST_RESULTS
    nc = _get_program()
    mask4, ident, invidx, ones1, mie, invtbc = _consts()

    qT = [np.ascontiguousarray(q[b].T).astype(NPBF) for b in range(B)]
    kT = [np.ascontiguousarray(k[b].T).astype(NPBF) for b in range(B)]
    vT = [np.ascontiguousarray(v[b].T).astype(NPBF) for b in range(B)]
    pT = [np.ascontiguousarray(p[b].T).astype(NPBF) for b in range(B)]
    wqc = wq_k.astype(NPBF)
    wkc = wk_k.astype(NPBF)
    wvc = wv_k.astype(NPBF)
    wcc = wc_k.astype(NPBF)

    in_maps = []
    for c in range(8):
        b, hg = c // 4, c % 4
        c0 = hg * DL
        wqb = np.ascontiguousarray(
            (wq_b[c0:c0 + DL].reshape(2, 128).T * NORM_D).astype(np.float32))
        wkb = np.ascontiguousarray(wk_b[c0:c0 + DL].reshape(2, 128).T.astype(np.float32))
        in_maps.append({
            "qT": qT[b], "kT": kT[b], "vT": vT[b],
            "pT": np.ascontiguousarray(pT[b][c0:c0 + DL]),
            "wq": np.ascontiguousarray(wqc[:, c0:c0 + DL]),
            "wk": np.ascontiguousarray(wkc[:, c0:c0 + DL]),
            "wv": np.ascontiguousarray(wvc[:, c0:c0 + DL]),
            "wc": np.ascontiguousarray(wcc[c0:c0 + DL, :]),
            "wqb": wqb, "wkb": wkb,
            "wvb": np.ascontiguousarray(wv_b[c0:c0 + DL].reshape(1, DL).astype(NPBF)),
            "ones1": ones1, "mask4": mask4, "ident": ident, "invidx": invidx,
            "maskinvE": mie, "invtbc": invtbc,
        })

    res = run_bass_kernel_spmd(
        nc, in_maps, core_ids=list(range(8)), trace=PROFILE)
    LAST_RESULTS = res

    out = np.zeros((B, S, DM), np.float32)
    for c in range(8):
        out[c // 4] += res.results[c]["out"]
    out += wc_b[None, None, :].astype(np.float32)
    return out

